# revision 7
# baseline (speedup 1.0000x reference)
"""Sparse Bahdanau attention kernel for Trainium2 (8 NeuronCores, data-parallel
over batch).

Shapes (hardcoded): B=32, S=2048, H=1024, QS=1024, VS=2048. Per core: 4 batches.

Math per batch b:
  q = query[b] @ Wq                                  # [H]
  scores[s] = sum_h v_energy[h] * tanh(q[h] + proj_key[b,s,h])
  alphas = softmax(scores masked by mask[b])         # [S]
  context = alphas @ value[b]                        # [VS]

Key idea (arch: sparse_attention): masked positions (mask==0, ~50% of S) get
alpha == 0 exactly, so their proj_key/value rows never need to leave HBM.
Per batch we build a compact index list of the mask==1 rows ON DEVICE, then
dma_gather only those rows. HBM traffic drops from 24MB to ~12.6MB per batch.

Index compaction (per batch, all tensors tiny):
  - mask loaded as [128, 16] with s = p + 128*i.
  - colsum[i] (over partitions) via ones-matmul; P[p,i] = inclusive prefix over
    partitions via a lower-triangular-ones matmul (host constant).
  - E_excl[i] = exclusive scan of colsum along free dim (tensor_tensor_scan).
  - C[p,i] = E_excl[i] + P[p,i] = inclusive global count; dest D = C-1 for
    mask==1, TRASH row for mask==0 (collision-free).
  - D is re-wrapped into the 16-partition index layout via an 8KB DRAM bounce,
    then dma_scatter_add writes payload s*m into a strided (256B rows) DRAM
    scratch pre-filled with -1: slot j holds row index, tail stays -1.
  - The list is loaded back 16-partition-wrapped and drives: pk gather, value
    gather (3 sub-gathers each, dynamic counts via registers), and the alphas
    scatter-back. Compact capacity NCAP=1152 (n_b ~ Binom(2048,.5) ~= 1024).

Compute in compact space ([128, 9] slots): ADD q (broadcast via
partition_broadcast) -> TANH -> MUL v_energy -> ACT-accumulate; softmax with
validity mask (slot < n_b); context via alphas-stationary f32r matmuls.

q is computed with f32r matmuls streamed over Wq chunks (full-rate PE).

A post-pass splits multi-wait instructions into chains of single-wait NOPs
(this walrus build fits only one sync-wait per instruction).
"""

import numpy as np

import concourse.bass as bass
import concourse.tile as tile
from concourse import mybir
from concourse import library_config
from concourse.bass_utils import run_bass_kernel_spmd

B, S, H, QS, VS = 32, 2048, 1024, 1024, 2048
NCORES = 8
BPC = B // NCORES  # batches per core

F32 = mybir.dt.float32
F32R = mybir.dt.float32r
I32 = mybir.dt.int32
I16 = mybir.dt.int16

SC16 = 16          # s-chunks of 128 in full space
NCH = 9            # compact chunks of 128
NCAP = NCH * 128   # compact capacity = 1152
CW = NCAP // 16    # idx columns in 16-partition wrap = 72
TRASH = NCAP       # trash row in idx scratch
IDXROWS = NCAP + 1
GG = 3             # chunks per gather group
NG = NCH // GG     # gather groups = 3

AF = mybir.ActivationFunctionType


# ---------------------------------------------------------------------------
# walrus single-wait post-pass (same as the dense baseline)
_SPLIT_TYPES = (
    "InstMatmult",
    "InstDMACopy",
    "InstActivation",
    "InstTensorCopy",
    "InstTensorTensor",
    "InstTensorReduce",
    "InstTensorScalarPtr",
    "InstMemset",
    "InstReciprocal",
    "InstLdweights",
    "InstDrain",
    "InstEventSemaphore",
    "InstNoOp",
    "InstDMAGatherAnt",
    "InstDMAScatterAddAnt",
    "InstPartitionBroadcast",
    "InstRegisterMemoryOperation",
)


def _make_wait_nop(nc, engine_type, wait):
    import bass_rust as _br

    bi = nc.engines[engine_type].nop(nofuse=True)
    sem = _br.SemaphoreHandle(wait.ant_name or f"sem{wait.id}", wait.id)
    bi._wait_ge(sem, wait.wait_value)
    ni = bi.ins
    for fn in nc.m.functions:
        for blk in fn.blocks:
            if blk.instructions and blk.instructions[-1].name == ni.name:
                lst = list(blk.instructions)
                lst.pop()
                blk.instructions = lst
                return ni
    raise RuntimeError("freshly added nop not found at any block tail")


def _split_excess_waits(nc):
    for fn in nc.m.functions:
        for blk in fn.blocks:
            offenders = [
                inst
                for inst in blk.instructions
                if inst.sync_info is not None
                and inst.sync_info.on_wait
                and len(inst.sync_info.on_wait) > 1
                and type(inst).__name__ in _SPLIT_TYPES
            ]
            if not offenders:
                continue
            pre = {}
            for inst in offenders:
                si = inst.sync_info
                waits = list(si.on_wait)
                pre[inst.name] = [
                    _make_wait_nop(nc, inst.engine, w) for w in waits[:-1]
                ]
                inst.sync_info = mybir.SyncInfo(
                    on_wait=[waits[-1]],
                    on_update=list(si.on_update) if si.on_update else [],
                )
            out = []
            for inst in blk.instructions:
                out.extend(pre.get(inst.name, ()))
                out.append(inst)
            blk.instructions = out
    return nc


def _ap(t, offset, dims):
    return bass.AP(tensor=t, offset=offset, ap=[list(d) for d in dims])


def _ap3(ap2):
    """Append a trailing [1, 1] dim to a 2D AP (for elem_size=1 DMA ops)."""
    return bass.AP(
        tensor=ap2.tensor,
        offset=ap2.offset,
        ap=[list(d) for d in ap2.ap] + [[1, 1]],
    )


def host_constants():
    ones128 = np.ones((128, 128), dtype=np.float32)
    mtri = (np.arange(128)[:, None] <= np.arange(128)[None, :]).astype(np.float32)
    # payload constant: s + 1 (the scatter ADDS onto the -1 prefill)
    iota16 = (
        1.0 + np.arange(128)[:, None] + 128.0 * np.arange(16)[None, :]
    ).astype(np.float32)
    iota9 = (
        np.arange(128)[:, None] + 128.0 * np.arange(NCH)[None, :]
    ).astype(np.float32)
    offs3 = np.array([[0.0, 384.0, 768.0]], dtype=np.float32)
    bsel = np.zeros((4, 4 * 128), dtype=np.float32)
    for b in range(4):
        bsel[b, b * 128 : (b + 1) * 128] = 1.0
    return {
        "cones": ones128,
        "cmtri": mtri,
        "ciota16": iota16,
        "ciota9": iota9,
        "coffs3": offs3,
        "cbsel": bsel,
    }


def build_nc():
    nc = bass.Bass()

    # Registers must be reserved before TileContext (it consumes the rest).
    # This walrus build cannot encode reg_load (InstTensorLoad: "ISA wrong
    # length"), so all DMA counts are static immediates in registers.
    reg2048 = nc.gpsimd.alloc_register("reg2048")
    reg1152 = nc.gpsimd.alloc_register("reg1152")
    reg384 = nc.gpsimd.alloc_register("reg384")

    query = nc.dram_tensor("query", [BPC, QS], F32, kind="ExternalInput")
    pk = nc.dram_tensor("proj_key", [BPC, S, H], F32, kind="ExternalInput")
    value = nc.dram_tensor("value", [BPC, S, VS], F32, kind="ExternalInput")
    mask = nc.dram_tensor("mask", [BPC, 1, S], I32, kind="ExternalInput")
    wq = nc.dram_tensor("Wq", [QS, H], F32, kind="ExternalInput")
    ve = nc.dram_tensor("v_energy", [H], F32, kind="ExternalInput")
    cones = nc.dram_tensor("cones", [128, 128], F32, kind="ExternalInput")
    cmtri = nc.dram_tensor("cmtri", [128, 128], F32, kind="ExternalInput")
    ciota16 = nc.dram_tensor("ciota16", [128, 16], F32, kind="ExternalInput")
    ciota9 = nc.dram_tensor("ciota9", [128, NCH], F32, kind="ExternalInput")
    coffs3 = nc.dram_tensor("coffs3", [1, 3], F32, kind="ExternalInput")
    cbsel = nc.dram_tensor("cbsel", [BPC, BPC * 128], F32, kind="ExternalInput")
    ctx_out = nc.dram_tensor("context", [BPC, 1, VS], F32, kind="ExternalOutput")
    al_out = nc.dram_tensor("alphas", [BPC, 1, S], F32, kind="ExternalOutput")

    with tile.TileContext(nc) as tc:
        nc.gpsimd.load_library(library_config.mlp)
        nc.gpsimd.reg_mov(reg2048, 2048)
        nc.gpsimd.reg_mov(reg1152, NCAP)
        nc.gpsimd.reg_mov(reg384, 384)
        with (
            tc.tile_pool(name="consts", bufs=1) as consts,
            tc.tile_pool(name="idxp", bufs=1) as idxp,
            tc.tile_pool(name="sm", bufs=2) as sm,
            tc.tile_pool(name="dramp", bufs=1, space="DRAM") as dramp,
        ):
            # ---------------- DRAM scratch ----------------
            d_bounce = dramp.tile([BPC, S], F32, tag="dbounce")
            idx_scr = dramp.tile([BPC, IDXROWS, 64], F32, tag="idxscr")
            al_scr = dramp.tile([BPC, S + 1, 64], F32, tag="alscr")

            # ---------------- constants ----------------
            ones_t = consts.tile([128, 128], F32)
            nc.sync.dma_start(out=ones_t, in_=_ap(cones, 0, [[128, 128], [1, 128]]))
            mtri_t = consts.tile([128, 128], F32)
            nc.sync.dma_start(out=mtri_t, in_=_ap(cmtri, 0, [[128, 128], [1, 128]]))
            iota16_t = consts.tile([128, 16], F32)
            nc.sync.dma_start(
                out=iota16_t, in_=_ap(ciota16, 0, [[16, 128], [1, 16]])
            )
            iota9_t = consts.tile([128, NCH], F32)
            nc.sync.dma_start(out=iota9_t, in_=_ap(ciota9, 0, [[NCH, 128], [1, NCH]]))
            offs3_t = consts.tile([1, 3], F32)
            nc.sync.dma_start(out=offs3_t, in_=_ap(coffs3, 0, [[3, 1], [1, 3]]))
            bsel_t = consts.tile([BPC, BPC * 128], F32R)
            nc.sync.dma_start(
                out=bsel_t,
                in_=_ap(
                    cbsel, 0, [[BPC * 128, BPC], [1, BPC * 128]]
                ).bitcast(F32R),
            )

            ve_bcast = consts.tile([128, H], F32)
            nc.gpsimd.dma_start(out=ve_bcast, in_=_ap(ve, 0, [[0, 128], [1, H]]))

            zerot = consts.tile([128, 16], F32)
            nc.vector.memset(zerot, 0.0)
            negt = consts.tile([16, CW], F32)
            nc.vector.memset(negt, -1.0)

            # al_scr col-0 zero fill + idx_scr -1 fill (per batch)
            for b in range(BPC):
                nc.gpsimd.dma_start(
                    out=_ap(
                        al_scr.tensor,
                        b * (S + 1) * 64,
                        [[64, 128], [64 * 128, 16]],
                    ),
                    in_=zerot,
                )
                nc.gpsimd.dma_start(
                    out=_ap(
                        idx_scr.tensor,
                        b * IDXROWS * 64,
                        [[64, 16], [16 * 64, CW]],
                    ),
                    in_=negt,
                )

            # ---------------- per-batch index pipeline ----------------
            mask_w = []
            for b in range(BPC):
                mw = idxp.tile([128, 16], I32, tag=f"mw{b}")
                nc.gpsimd.dma_start(
                    out=mw, in_=_ap(mask, b * S, [[1, 128], [128, 16]])
                )
                mask_w.append(mw)

            nb_f, valid_t, gidx_t, gidxa_t = [], [], [], []
            with tc.tile_pool(name="pidx", bufs=2, space="PSUM") as pidx:
                for b in range(BPC):
                    m_f = idxp.tile([128, 16], F32, tag=f"mf{b}")
                    nc.vector.tensor_copy(out=m_f, in_=mask_w[b])

                    cs_ps = pidx.tile([128, 16], F32, tag="cs")
                    nc.tensor.matmul(
                        out=cs_ps, lhsT=ones_t, rhs=m_f, start=True, stop=True
                    )
                    pp_ps = pidx.tile([128, 16], F32, tag="pp")
                    nc.tensor.matmul(
                        out=pp_ps, lhsT=mtri_t, rhs=m_f, start=True, stop=True
                    )

                    cs_sb = idxp.tile([128, 16], F32, tag=f"cs{b}")
                    nc.vector.tensor_copy(out=cs_sb, in_=cs_ps)

                    einc = idxp.tile([128, 16], F32, tag=f"ei{b}")
                    nc.vector.tensor_tensor_scan(
                        out=einc,
                        data0=cs_sb,
                        data1=cs_sb,
                        initial=0.0,
                        op0=mybir.AluOpType.add,
                        op1=mybir.AluOpType.bypass,
                    )
                    # C = (einc - cs) + P   (global inclusive count)
                    cfull = idxp.tile([128, 16], F32, tag=f"cf{b}")
                    nc.vector.tensor_sub(out=cfull, in0=einc, in1=cs_sb)
                    nc.vector.tensor_add(out=cfull, in0=cfull, in1=pp_ps)

                    # n_b on every partition
                    nbf = idxp.tile([128, 1], F32, tag=f"nb{b}")
                    nc.vector.reduce_sum(
                        out=nbf, in_=cs_sb, axis=mybir.AxisListType.X
                    )
                    nb_f.append(nbf)

                    # D = m * (C - 1 - TRASH) + TRASH
                    dfull = idxp.tile([128, 16], F32, tag=f"df{b}")
                    nc.vector.tensor_scalar_add(
                        out=dfull, in0=cfull, scalar1=float(-1 - TRASH)
                    )
                    nc.vector.tensor_mul(out=dfull, in0=dfull, in1=m_f)
                    nc.vector.tensor_scalar_add(
                        out=dfull, in0=dfull, scalar1=float(TRASH)
                    )

                    # payload = s * m
                    payload = idxp.tile([128, 16], F32, tag=f"pl{b}")
                    nc.vector.tensor_mul(out=payload, in0=iota16_t, in1=m_f)

                    # bounce D: [128,16] (s = p+128i) -> DRAM -> [16,128] wrap
                    nc.gpsimd.dma_start(
                        out=_ap(d_bounce.tensor, b * S, [[1, 128], [128, 16]]),
                        in_=dfull,
                    )
                    # the Pool Q7 cpus each read the idx list from their own
                    # 16-partition group: replicate it across all 8 groups
                    d_bw = idxp.tile([128, 128], F32, tag=f"dbw{b}")
                    for g8 in range(8):
                        nc.gpsimd.dma_start(
                            out=d_bw[16 * g8 : 16 * (g8 + 1), :],
                            in_=_ap(
                                d_bounce.tensor, b * S, [[1, 16], [16, 128]]
                            ),
                        )
                    d_w16 = idxp.tile([128, 128], I16, tag=f"dw{b}")
                    nc.vector.tensor_copy(out=d_w16, in_=d_bw)

                    # scatter payload into idx scratch (strided 256B rows)
                    nc.gpsimd.dma_scatter_add(
                        out_ap=_ap(
                            idx_scr.tensor,
                            b * IDXROWS * 64,
                            [[64, IDXROWS], [1, 1]],
                        ),
                        in_ap=_ap3(payload[:, :]),
                        idxs_ap=d_w16[:, :],
                        num_idxs=2048,
                        num_idxs_reg=reg2048,
                        elem_size=1,
                        elem_step=64,
                    )

                    # load back the wrapped index list, replicated across
                    # all eight 16-partition groups (per-Q7-cpu reads)
                    gidx_f = idxp.tile([128, CW], F32, tag=f"gf{b}")
                    for g8 in range(8):
                        nc.gpsimd.dma_start(
                            out=gidx_f[16 * g8 : 16 * (g8 + 1), :],
                            in_=_ap(
                                idx_scr.tensor,
                                b * IDXROWS * 64,
                                [[64, 16], [16 * 64, CW]],
                            ),
                        )
                    # gather variant: -1 pads -> row 0 (duplicate reads ok)
                    g0f = idxp.tile([128, CW], F32, tag=f"g0f{b}")
                    nc.vector.tensor_scalar_max(out=g0f, in0=gidx_f, scalar1=0.0)
                    gidx = idxp.tile([128, CW], I16, tag=f"gi{b}")
                    nc.vector.tensor_copy(out=gidx, in_=g0f)
                    gidx_t.append(gidx)
                    # alphas-scatter variant: -1 pads -> trash row S
                    negf = idxp.tile([128, CW], F32, tag=f"ng{b}")
                    nc.vector.tensor_scalar(
                        out=negf,
                        in0=gidx_f,
                        scalar1=0.0,
                        scalar2=None,
                        op0=mybir.AluOpType.is_lt,
                    )
                    nc.vector.tensor_scalar_mul(
                        out=negf, in0=negf, scalar1=float(S + 1)
                    )
                    nc.vector.tensor_add(out=negf, in0=negf, in1=gidx_f)
                    gidxa = idxp.tile([128, CW], I16, tag=f"ga{b}")
                    nc.vector.tensor_copy(out=gidxa, in_=negf)
                    gidxa_t.append(gidxa)

                    # validity mask in compact space
                    vld = idxp.tile([128, NCH], F32, tag=f"vl{b}")
                    nc.vector.tensor_scalar(
                        out=vld,
                        in0=iota9_t,
                        scalar1=nbf[:, 0:1],
                        scalar2=None,
                        op0=mybir.AluOpType.is_lt,
                    )
                    valid_t.append(vld)



            # ---------------- q = query @ Wq (f32r, streamed) ----------
            qT = consts.tile([128, 8, BPC], F32R)
            for k in range(8):
                nc.sync.dma_start(
                    out=qT[:, k, :],
                    in_=_ap(query, k * 128, [[1, 128], [QS, BPC]]).bitcast(
                        F32R
                    ),
                )
            q_sb = consts.tile([BPC, H], F32R)
            with (
                tc.tile_pool(name="wqp", bufs=2) as wqp,
                tc.tile_pool(name="pq", bufs=1, space="PSUM") as pq,
            ):
                q_ps = []
                for h in range(2):
                    qh = pq.tile([BPC, 512], F32, tag=f"qh{h}", name=f"qh{h}")
                    q_ps.append(qh)
                for k in range(8):
                    wqc = wqp.tile([128, H], F32R)
                    nc.sync.dma_start(
                        out=wqc,
                        in_=_ap(wq, k * 128 * H, [[H, 128], [1, H]]).bitcast(
                            F32R
                        ),
                    )
                    for h in range(2):
                        nc.tensor.matmul(
                            out=q_ps[h],
                            lhsT=qT[:, k, :],
                            rhs=wqc[:, h * 512 : (h + 1) * 512],
                            start=(k == 0),
                            stop=(k == 7),
                            skip_group_check=True,
                        )
                for h in range(2):
                    nc.vector.tensor_copy(
                        out=q_sb[:, h * 512 : (h + 1) * 512], in_=q_ps[h]
                    )

            # broadcast q[b] to all 128 partitions: one-hot selector matmul
            q_bc = []
            with tc.tile_pool(name="pqbc", bufs=2, space="PSUM") as pqbc:
                for b in range(BPC):
                    qbp = pqbc.tile([128, H], F32, tag="qb", name=f"qbp{b}")
                    for h in range(2):
                        nc.tensor.matmul(
                            out=qbp[:, h * 512 : (h + 1) * 512],
                            lhsT=bsel_t[:, b * 128 : (b + 1) * 128],
                            rhs=q_sb[:, h * 512 : (h + 1) * 512],
                            start=True,
                            stop=True,
                            skip_group_check=True,
                        )
                    qb = consts.tile([128, H], F32, tag=f"qbc{b}", name=f"qb{b}")
                    nc.vector.tensor_copy(out=qb, in_=qbp)
                    q_bc.append(qb)

            # ---------------- main loop ----------------
            ctx_sb = []
            for b in range(BPC):
                cxt = consts.tile([1, VS], F32, tag=f"ctx{b}", name=f"ctxsb{b}")
                ctx_sb.append(cxt)
            with (
                tc.tile_pool(name="pkc", bufs=3) as pkc,
                tc.tile_pool(name="vc", bufs=3) as vc,
                tc.tile_pool(name="tp", bufs=3) as tp,
                tc.tile_pool(name="psz", bufs=2, space="PSUM") as psz,
                tc.tile_pool(name="psctx", bufs=1, space="PSUM") as psctx,
            ):
                for b in range(BPC):
                    scores_b = sm.tile([128, NCH], F32, tag="scores")
                    pk_src = _ap(pk, b * S * H, [[H, S], [1, H]])
                    for g in range(NG):
                        pkt = pkc.tile([128, GG, H], F32, tag="pkt")
                        if g == NG - 1:
                            # slots >= n_b live only in the last two compact
                            # chunks (n_b >= 897 guaranteed by the host guard)
                            nc.vector.memset(pkt[:, 1:GG, :], 0.0)
                        nc.gpsimd.dma_gather(
                            out_ap=pkt[:, :, :],
                            in_ap=pk_src,
                            idxs_ap=gidx_t[b][:, g * 24 : (g + 1) * 24],
                            num_idxs=384,
                            num_idxs_reg=reg384,
                            elem_size=H,
                        )
                        for cc in range(GG):
                            c = g * GG + cc
                            t_t = tp.tile([128, H], F32)
                            nc.vector.tensor_add(
                                out=t_t, in0=pkt[:, cc, :], in1=q_bc[b]
                            )
                            nc.scalar.activation(out=t_t, in_=t_t, func=AF.Tanh)
                            nc.vector.tensor_mul(out=t_t, in0=t_t, in1=ve_bcast)
                            nc.scalar.activation(
                                out=t_t,
                                in_=t_t,
                                func=AF.Identity,
                                accum_out=scores_b[:, c : c + 1],
                            )

                    # masked softmax in compact space
                    e_t = sm.tile([128, NCH], F32, tag="e")
                    nc.scalar.activation(out=e_t, in_=scores_b, func=AF.Exp)
                    nc.vector.tensor_mul(out=e_t, in0=e_t, in1=valid_t[b])
                    rowsum = sm.tile([128, 1], F32, tag="rowsum")
                    nc.vector.reduce_sum(
                        out=rowsum, in_=e_t, axis=mybir.AxisListType.X
                    )
                    zp = psz.tile([128, 1], F32, tag="z")
                    nc.tensor.matmul(
                        out=zp, lhsT=ones_t, rhs=rowsum, start=True, stop=True
                    )
                    recip = sm.tile([128, 1], F32, tag="recip")
                    nc.vector.tensor_copy(out=recip, in_=zp)
                    nc.vector.reciprocal(out=recip, in_=recip)

                    alphas_c = idxp.tile([128, NCH, 1], F32, tag=f"al{b}")
                    nc.vector.tensor_scalar_mul(
                        out=alphas_c[:, :, 0], in0=e_t, scalar1=recip
                    )
                    alphas_r = sm.tile([128, NCH], F32R, tag="alr")
                    nc.vector.tensor_copy(out=alphas_r, in_=alphas_c[:, :, 0])

                    # context: gather value rows, alphas-stationary matmul
                    ctxp = psctx.tile([1, VS], F32, tag="ctx")
                    v_src = _ap(value, b * S * VS, [[VS, S], [1, VS]]).bitcast(
                        F32R
                    )
                    for g in range(NG):
                        vt = vc.tile([128, GG, VS], F32R, tag="vt")
                        if g == NG - 1:
                            nc.vector.memset(vt[:, 1:GG, :].bitcast(F32), 0.0)
                        nc.gpsimd.dma_gather(
                            out_ap=vt[:, :, :],
                            in_ap=v_src,
                            idxs_ap=gidx_t[b][:, g * 24 : (g + 1) * 24],
                            num_idxs=384,
                            num_idxs_reg=reg384,
                            elem_size=VS,
                        )
                        for cc in range(GG):
                            c = g * GG + cc
                            for j in range(VS // 512):
                                nc.tensor.matmul(
                                    out=ctxp[0:1, j * 512 : (j + 1) * 512],
                                    lhsT=alphas_r[:, c : c + 1],
                                    rhs=vt[:, cc, j * 512 : (j + 1) * 512],
                                    start=(c == 0),
                                    stop=(c == NCH - 1),
                                    skip_group_check=True,
                                )
                    nc.vector.tensor_copy(out=ctx_sb[b], in_=ctxp)

                    # alphas out: scatter compact -> strided scratch -> al_out
                    nc.gpsimd.dma_scatter_add(
                        out_ap=_ap(
                            al_scr.tensor,
                            b * (S + 1) * 64,
                            [[64, S + 1], [1, 1]],
                        ),
                        in_ap=alphas_c[:, :, :],
                        idxs_ap=gidxa_t[b][:, :],
                        num_idxs=NCAP,
                        num_idxs_reg=reg1152,
                        elem_size=1,
                        elem_step=64,
                    )
                    alf = idxp.tile([128, 16], F32, tag=f"alf{b}")
                    nc.gpsimd.dma_start(
                        out=alf,
                        in_=_ap(
                            al_scr.tensor,
                            b * (S + 1) * 64,
                            [[64, 128], [64 * 128, 16]],
                        ),
                    )
                    nc.gpsimd.dma_start(
                        out=_ap(al_out, b * S, [[1, 128], [128, 16]]), in_=alf
                    )

                for b in range(BPC):
                    nc.gpsimd.dma_start(
                        out=_ap(ctx_out, b * VS, [[VS, 1], [1, VS]]),
                        in_=ctx_sb[b],
                    )

    _split_excess_waits(nc)
    # Raw Bass skips Bacc's extended-inst codegen; without it the NEFF
    # compiler sees empty .instr on bass_isa InstISA subclasses (e.g. the
    # library reload) and dies with "ISA wrong length".
    mybir.codegen_inst_isa_subclasses(nc)
    return nc


_NC_CACHE = None


def _get_nc():
    global _NC_CACHE
    if _NC_CACHE is None:
        _NC_CACHE = build_nc()
    return _NC_CACHE


def kernel(query, proj_key, value, mask, Wq, v_energy, _want_results_obj=False,
           _trace=False):
    query = np.asarray(query, dtype=np.float32)
    proj_key = np.asarray(proj_key, dtype=np.float32)
    value = np.asarray(value, dtype=np.float32)
    mask = np.asarray(mask, dtype=np.int32)
    Wq = np.asarray(Wq, dtype=np.float32)
    v_energy = np.asarray(v_energy, dtype=np.float32)

    # Safety net for pathological masks (never triggers for Bernoulli(0.5)
    # masks at S=2048: bounds are >5 sigma out). The compact capacity NCAP and
    # the tail-chunk memsets assume 897 <= n_b <= 1152 per batch.
    nbs = (mask != 0).sum(axis=(1, 2))
    if nbs.min() < 897 or nbs.max() > NCAP:
        q = (query @ Wq)[:, None, :]
        scores = np.einsum(
            "bsh,h->bs", np.tanh(q + proj_key), v_energy
        )[:, None, :].astype(np.float32)
        scores = np.where(mask == 0, -np.inf, scores)
        m = scores.max(axis=-1, keepdims=True)
        e = np.exp(scores - m)
        alphas = (e / e.sum(axis=-1, keepdims=True)).astype(np.float32)
        context = np.matmul(alphas, value).astype(np.float32)
        if _want_results_obj:
            return (context, alphas), None
        return context, alphas

    consts = host_constants()
    nc = _get_nc()
    in_maps = []
    for k in range(NCORES):
        sl = slice(k * BPC, (k + 1) * BPC)
        m = {
            "query": query[sl],
            "proj_key": proj_key[sl],
            "value": value[sl],
            "mask": mask[sl],
            "Wq": Wq,
            "v_energy": v_energy,
        }
        m.update(consts)
        in_maps.append(m)
    res = run_bass_kernel_spmd(
        nc, in_maps, core_ids=list(range(NCORES)), trace=_trace
    )
    ctx = np.concatenate([r["context"] for r in res.results], axis=0)
    al = np.concatenate([r["alphas"] for r in res.results], axis=0)
    if _want_results_obj:
        return (ctx, al), res
    return ctx, al


# revision 10
# speedup vs baseline: 2.8574x; 2.8574x over previous
"""Dense Bahdanau attention kernel for Trainium2 (8 NeuronCores, data-parallel
over batch) — pipelined v2.

Shapes (hardcoded): B=32, S=2048, H=1024, QS=1024, VS=2048. Per core: 4 batches.

Differences vs the 381us baseline (trace-driven):
  - q = query @ Wq computed with f32r matmuls streamed over 512KB Wq chunks
    (full-rate PE, ~6us instead of ~24us fp32), and q broadcast to 128
    partitions with a one-hot selector matmul instead of a DRAM bounce.
  - Streaming pools (pk/value) no longer reuse the Wq SBUF region, so the
    bulk loads start at t=0 instead of waiting for the q computation.
  - The per-batch context psum->sbuf copy moved from DVE to GPSIMD: the DVE
    program order was serializing batch N's context matmul (i.e. its full
    value load) against batch N+1's scores - ~20us of DMA idle per batch.
  - value streamed in 2MB groups with 5 buffers (finer rotation smooths the
    load/consume pipeline and shrinks the tail).
"""

import numpy as np

import concourse.bass as bass
import concourse.tile as tile
from concourse import mybir
from concourse.bass_utils import run_bass_kernel_spmd

B, S, H, QS, VS = 32, 2048, 1024, 1024, 2048
NCORES = 8
BPC = B // NCORES  # batches per core

F32 = mybir.dt.float32
F32R = mybir.dt.float32r
I32 = mybir.dt.int32

SC = S // 128  # 16 s-chunks of 128
PKG = 4        # s-chunks per proj_key DMA (2MB)
VG = 2         # s-chunks per value DMA (2MB)

AF = mybir.ActivationFunctionType

_SPLIT_TYPES = (
    "InstMatmult",
    "InstDMACopy",
    "InstActivation",
    "InstTensorCopy",
    "InstTensorTensor",
    "InstTensorReduce",
    "InstTensorScalarPtr",
    "InstMemset",
    "InstReciprocal",
    "InstLdweights",
    "InstDrain",
    "InstEventSemaphore",
    "InstNoOp",
)


def _make_wait_nop(nc, engine_type, wait):
    import bass_rust as _br

    bi = nc.engines[engine_type].nop(nofuse=True)
    sem = _br.SemaphoreHandle(wait.ant_name or f"sem{wait.id}", wait.id)
    bi._wait_ge(sem, wait.wait_value)
    ni = bi.ins
    for fn in nc.m.functions:
        for blk in fn.blocks:
            if blk.instructions and blk.instructions[-1].name == ni.name:
                lst = list(blk.instructions)
                lst.pop()
                blk.instructions = lst
                return ni
    raise RuntimeError("freshly added nop not found at any block tail")


def _split_excess_waits(nc):
    for fn in nc.m.functions:
        for blk in fn.blocks:
            offenders = [
                inst
                for inst in blk.instructions
                if inst.sync_info is not None
                and inst.sync_info.on_wait
                and len(inst.sync_info.on_wait) > 1
                and type(inst).__name__ in _SPLIT_TYPES
            ]
            if not offenders:
                continue
            pre = {}
            for inst in offenders:
                si = inst.sync_info
                waits = list(si.on_wait)
                pre[inst.name] = [
                    _make_wait_nop(nc, inst.engine, w) for w in waits[:-1]
                ]
                inst.sync_info = mybir.SyncInfo(
                    on_wait=[waits[-1]],
                    on_update=list(si.on_update) if si.on_update else [],
                )
            out = []
            for inst in blk.instructions:
                out.extend(pre.get(inst.name, ()))
                out.append(inst)
            blk.instructions = out
    return nc


def _ap(t, offset, dims):
    return bass.AP(tensor=t, offset=offset, ap=[list(d) for d in dims])


def host_constants():
    bsel = np.zeros((4, 4 * 128), dtype=np.float32)
    for b in range(4):
        bsel[b, b * 128 : (b + 1) * 128] = 1.0
    return {"cbsel": bsel}


def build_nc():
    nc = bass.Bass()

    query = nc.dram_tensor("query", [BPC, QS], F32, kind="ExternalInput")
    pk = nc.dram_tensor("proj_key", [BPC, S, H], F32, kind="ExternalInput")
    value = nc.dram_tensor("value", [BPC, S, VS], F32, kind="ExternalInput")
    mask = nc.dram_tensor("mask", [BPC, 1, S], I32, kind="ExternalInput")
    wq = nc.dram_tensor("Wq", [QS, H], F32, kind="ExternalInput")
    ve = nc.dram_tensor("v_energy", [H], F32, kind="ExternalInput")
    cbsel = nc.dram_tensor("cbsel", [BPC, BPC * 128], F32, kind="ExternalInput")
    ctx_out = nc.dram_tensor("context", [BPC, 1, VS], F32, kind="ExternalOutput")
    al_out = nc.dram_tensor("alphas", [BPC, 1, S], F32, kind="ExternalOutput")

    with tile.TileContext(nc) as tc:
        with (
            tc.tile_pool(name="consts", bufs=1) as consts,
            tc.tile_pool(name="pkp", bufs=3) as pkp,
            tc.tile_pool(name="vp", bufs=5) as vp,
            tc.tile_pool(name="tp", bufs=3) as tp,
            tc.tile_pool(name="sm", bufs=2) as sm,
        ):
            # ---- constants / prologue ----
            ve_bcast = consts.tile([128, H], F32)
            nc.gpsimd.dma_start(out=ve_bcast, in_=_ap(ve, 0, [[0, 128], [1, H]]))

            ones128 = consts.tile([128, 128], F32)
            nc.vector.memset(ones128, 1.0)

            bsel_t = consts.tile([BPC, BPC * 128], F32)
            nc.gpsimd.dma_start(
                out=bsel_t,
                in_=_ap(cbsel, 0, [[BPC * 128, BPC], [1, BPC * 128]]),
            )

            # q = query @ Wq in f32r, streamed over 512KB Wq chunks
            qT = consts.tile([128, 8, BPC], F32R)
            for k in range(8):
                nc.sync.dma_start(
                    out=qT[:, k, :],
                    in_=_ap(query, k * 128, [[1, 128], [QS, BPC]]).bitcast(F32R),
                )
            q_sb = consts.tile([BPC, H], F32)
            with (
                tc.tile_pool(name="wqp", bufs=2) as wqp,
                tc.tile_pool(name="pq", bufs=1, space="PSUM") as pq,
            ):
                q_ps = []
                for h in range(2):
                    qh = pq.tile([BPC, 512], F32, tag=f"qh{h}", name=f"qh{h}")
                    q_ps.append(qh)
                for k in range(8):
                    wqc = wqp.tile([128, H], F32R)
                    nc.sync.dma_start(
                        out=wqc,
                        in_=_ap(wq, k * 128 * H, [[H, 128], [1, H]]).bitcast(
                            F32R
                        ),
                    )
                    for h in range(2):
                        nc.tensor.matmul(
                            out=q_ps[h],
                            lhsT=qT[:, k, :],
                            rhs=wqc[:, h * 512 : (h + 1) * 512],
                            start=(k == 0),
                            stop=(k == 7),
                            skip_group_check=True,
                        )
                for h in range(2):
                    nc.vector.tensor_copy(
                        out=q_sb[:, h * 512 : (h + 1) * 512], in_=q_ps[h]
                    )

            # broadcast q[b] to 128 partitions via one-hot selector matmul
            q_bc = []
            with tc.tile_pool(name="pqbc", bufs=2, space="PSUM") as pqbc:
                for b in range(BPC):
                    qbp = pqbc.tile([128, H], F32, tag="qb", name=f"qbp{b}")
                    for h in range(2):
                        nc.tensor.matmul(
                            out=qbp[:, h * 512 : (h + 1) * 512],
                            lhsT=bsel_t[:, b * 128 : (b + 1) * 128],
                            rhs=q_sb[:, h * 512 : (h + 1) * 512],
                            start=True,
                            stop=True,
                            skip_group_check=True,
                        )
                    qb = consts.tile([128, H], F32, tag=f"qbc{b}", name=f"qb{b}")
                    nc.vector.tensor_copy(out=qb, in_=qbp)
                    q_bc.append(qb)

            # ---- main loop over batches ----
            with (
                tc.tile_pool(name="psz", bufs=2, space="PSUM") as psz,
                tc.tile_pool(name="psctx", bufs=1, space="PSUM") as psctx,
            ):
                pending_ctx = [None] * BPC
                for b in range(BPC):
                    # scores phase: [128 s, 16 cols]
                    scores_b = sm.tile([128, SC], F32, tag="scores")
                    for g in range(SC // PKG):
                        pk_t = pkp.tile([128, PKG, H], F32)
                        nc.scalar.dma_start(
                            out=pk_t,
                            in_=_ap(
                                pk,
                                b * S * H + g * PKG * 128 * H,
                                [[H, 128], [128 * H, PKG], [1, H]],
                            ),
                        )
                        for cc in range(PKG):
                            c = g * PKG + cc
                            t_t = tp.tile([128, H], F32)
                            nc.vector.tensor_add(
                                out=t_t, in0=pk_t[:, cc, :], in1=q_bc[b]
                            )
                            nc.scalar.activation(out=t_t, in_=t_t, func=AF.Tanh)
                            nc.vector.tensor_mul(out=t_t, in0=t_t, in1=ve_bcast)
                            nc.scalar.activation(
                                out=t_t,
                                in_=t_t,
                                func=AF.Identity,
                                accum_out=scores_b[:, c : c + 1],
                            )

                    # masked softmax (no max subtraction; |scores| <= ~26)
                    mask_i = sm.tile([128, SC], I32, tag="mask_i")
                    nc.gpsimd.dma_start(
                        out=mask_i, in_=_ap(mask, b * S, [[1, 128], [128, SC]])
                    )
                    mask_f = sm.tile([128, SC], F32, tag="mask_f")
                    nc.vector.tensor_copy(out=mask_f, in_=mask_i)

                    e_t = sm.tile([128, SC], F32, tag="e")
                    nc.scalar.activation(out=e_t, in_=scores_b, func=AF.Exp)
                    nc.vector.tensor_mul(out=e_t, in0=e_t, in1=mask_f)
                    rowsum = sm.tile([128, 1], F32, tag="rowsum")
                    nc.vector.reduce_sum(
                        out=rowsum, in_=e_t, axis=mybir.AxisListType.X
                    )

                    zp = psz.tile([128, 1], F32, tag="z")
                    nc.tensor.matmul(
                        out=zp, lhsT=ones128, rhs=rowsum, start=True, stop=True
                    )
                    recip = sm.tile([128, 1], F32, tag="recip")
                    nc.vector.tensor_copy(out=recip, in_=zp)
                    nc.vector.reciprocal(out=recip, in_=recip)

                    alphas_t = sm.tile([128, SC], F32, tag="alphas")
                    nc.vector.tensor_scalar_mul(
                        out=alphas_t, in0=e_t, scalar1=recip
                    )
                    nc.gpsimd.dma_start(
                        out=_ap(al_out, b * S, [[1, 128], [128, SC]]),
                        in_=alphas_t,
                    )

                    # context phase: alphas-stationary f32r matmuls
                    alphas_r = sm.tile([128, SC], F32R, tag="alphas_r")
                    nc.vector.tensor_copy(out=alphas_r, in_=alphas_t)

                    # drain batch b-1's context now: its matmuls finished
                    # while batch b's scores were computing, so this DVE copy
                    # does not stall the engine (and it frees the ctx psum
                    # right before batch b's first accumulation)
                    if pending_ctx[b - 1] is not None:
                        prev_b, prev_ctxp = pending_ctx[b - 1]
                        ctx_sb = sm.tile([1, VS], F32, tag="ctx_sb")
                        nc.vector.tensor_copy(out=ctx_sb, in_=prev_ctxp)
                        nc.gpsimd.dma_start(
                            out=_ap(ctx_out, prev_b * VS, [[VS, 1], [1, VS]]),
                            in_=ctx_sb,
                        )
                        pending_ctx[b - 1] = None
                    ctxp = psctx.tile([1, VS], F32, tag="ctx")
                    for g in range(SC // VG):
                        v_t = vp.tile([128, VG, VS], F32R)
                        nc.sync.dma_start(
                            out=v_t,
                            in_=_ap(
                                value,
                                b * S * VS + g * VG * 128 * VS,
                                [[VS, 128], [128 * VS, VG], [1, VS]],
                            ).bitcast(F32R),
                        )
                        for cc in range(VG):
                            c = g * VG + cc
                            for j in range(VS // 512):
                                nc.tensor.matmul(
                                    out=ctxp[0:1, j * 512 : (j + 1) * 512],
                                    lhsT=alphas_r[:, c : c + 1],
                                    rhs=v_t[:, cc, j * 512 : (j + 1) * 512],
                                    start=(c == 0),
                                    stop=(c == SC - 1),
                                    skip_group_check=True,
                                )
                    pending_ctx[b] = (b, ctxp)

                # drain the last batch's context
                last_b, last_ctxp = pending_ctx[BPC - 1]
                ctx_sb = sm.tile([1, VS], F32, tag="ctx_sb")
                nc.vector.tensor_copy(out=ctx_sb, in_=last_ctxp)
                nc.gpsimd.dma_start(
                    out=_ap(ctx_out, last_b * VS, [[VS, 1], [1, VS]]),
                    in_=ctx_sb,
                )

    return _split_excess_waits(nc)


_NC_CACHE = None


def _get_nc():
    global _NC_CACHE
    if _NC_CACHE is None:
        _NC_CACHE = build_nc()
    return _NC_CACHE


def kernel(query, proj_key, value, mask, Wq, v_energy, _want_results_obj=False,
           _trace=False):
    query = np.asarray(query, dtype=np.float32)
    proj_key = np.asarray(proj_key, dtype=np.float32)
    value = np.asarray(value, dtype=np.float32)
    mask = np.asarray(mask, dtype=np.int32)
    Wq = np.asarray(Wq, dtype=np.float32)
    v_energy = np.asarray(v_energy, dtype=np.float32)

    consts = host_constants()
    nc = _get_nc()
    in_maps = []
    for k in range(NCORES):
        sl = slice(k * BPC, (k + 1) * BPC)
        m = {
            "query": query[sl],
            "proj_key": proj_key[sl],
            "value": value[sl],
            "mask": mask[sl],
            "Wq": Wq,
            "v_energy": v_energy,
        }
        m.update(consts)
        in_maps.append(m)
    res = run_bass_kernel_spmd(
        nc, in_maps, core_ids=list(range(NCORES)), trace=_trace
    )
    ctx = np.concatenate([r["context"] for r in res.results], axis=0)
    al = np.concatenate([r["alphas"] for r in res.results], axis=0)
    if _want_results_obj:
        return (ctx, al), res
    return ctx, al
